# revision 49
# baseline (speedup 1.0000x reference)
"""Causal single-head attention on 8 TRN2 NeuronCores (Bass/Tile).

Problem: x[4,4096,1024] @ {Wq,Wk,Wv}[1024,64] (+zero biases) -> causal
softmax attention -> out[4,4096,64], fp32.

Sharding: 8 cores = 4 batches x 2 parities. Each core owns 4 query
blocks of 512 rows. Parity-1 cores receive x^T rolled left by 512
columns so every core's query blocks sit at uniform offsets 1024*i,
keeping the program SPMD-identical; causality is enforced by 4
data-driven diagonal mask tiles plus a parity-dependent pre-exp bias
(-1e30 kills the wrap-around key tiles on parity-0 cores).

Softmax uses no max-subtraction (scores ~N(0,0.25^2), exp is safe) and
the denominator comes from a ones-column appended to V, so there are no
cross-partition reductions. All matmuls run as float32r (full PE rate
at N>=256 moving dim; rel err ~2e-4).

Schedule: all x^T column-block DMAs are issued up-front (SP queue runs
them back-to-back at full DMA rate) in stream order 0,2,4,6,7,1,3,5;
blocks arrive in 1-2 chunk pieces. Attention work is queued as micro-op
thunks (one S->exp->mask or one AV each) and interleaved between the
projection chunk matmuls, so the PE fills the 515ns DMA-trickle gaps
and the ScalarE exp stream never starves during a stream window. The S
matmul runs LOOK=3 key-tiles ahead of the AV accumulation; diagonal
tiles are S-emitted one phase early so their GPSIMD mask-multiply is
off the critical path. PSUM banks: pkv 1 + shared{pq,transpose} 2 +
S 4 + attention-out po 2 = 8 (projection and transpose PSUM share
one 2-buffer pool: their lifetimes chain through the same kv_s drain).
"""

import numpy as np

B, T, D, H = 4, 4096, 1024, 64
NCORES = 8
QB = 512          # query block width (free dim of attention matmuls)
KT = 128          # key tile (partition dim of P^T)
DC = D // 128     # 8 contraction chunks
CB = 512          # x^T column block for streaming
NCB = T // CB     # 8
NKT = T // KT     # 32
NB = 4            # query blocks per core
HE = H + 1        # V extended with a ones column (softmax denominator)

_PROGRAM = None


def _build_program():
    from contextlib import ExitStack

    import concourse.bass as bass  # noqa: F401
    import concourse.mybir as mybir
    import concourse.tile as tile
    from concourse import bacc
    from concourse.masks import make_identity

    f32 = mybir.dt.float32
    f32r = mybir.dt.float32r
    AF = mybir.ActivationFunctionType

    nc = bacc.Bacc(target_bir_lowering=False)
    xt_d = nc.dram_tensor("xt", [D, T], f32r, kind="ExternalInput").ap()
    wq_d = nc.dram_tensor("wq", [128, DC * H], f32r, kind="ExternalInput").ap()
    wkv_d = nc.dram_tensor("wkv", [D, 2 * H], f32r, kind="ExternalInput").ap()
    # col 0 rows 0:64 = bq, col 1 = bkv, col 2 = tail bias (0 / -1e30)
    bias_d = nc.dram_tensor("bias", [128, 3], f32, kind="ExternalInput").ap()
    on_d = nc.dram_tensor("ones", [128, NKT], f32r, kind="ExternalInput").ap()
    o_d = nc.dram_tensor("o", [NB * 2, 128, 2 * H], f32, kind="ExternalOutput").ap()

    with ExitStack() as ctx:
        tc = ctx.enter_context(tile.TileContext(nc))
        const = ctx.enter_context(tc.tile_pool(name="const", bufs=1))
        xt_pool = ctx.enter_context(tc.tile_pool(name="xtp", bufs=5))
        ppool = ctx.enter_context(tc.tile_pool(name="ptp", bufs=16))
        opool = ctx.enter_context(tc.tile_pool(name="otp", bufs=2))
        ps_a = ctx.enter_context(tc.tile_pool(name="psA", bufs=1, space="PSUM"))
        ps_qt = ctx.enter_context(tc.tile_pool(name="psQT", bufs=2, space="PSUM"))
        ps_s = ctx.enter_context(tc.tile_pool(name="psS", bufs=2, space="PSUM"))
        ps_o = ctx.enter_context(tc.tile_pool(name="psO", bufs=1, space="PSUM"))

        # Persistent SBUF state
        wq_s = const.tile([128, DC * H], f32r)        # chunk d at cols d*H
        wkv_s = const.tile([128, DC * 2 * H], f32r)   # chunk d at cols d*2H
        bias_s = const.tile([128, 3], f32)
        bq_s = bias_s[0:H, 0:1]
        bkv_s = bias_s[:, 1:2]
        tb_s = bias_s[:, 2:3]                        # tail-tile exp bias
        mk_s = const.tile([KT, 4 * QB], f32)          # mask slot s at cols s*QB
        zb_s = const.tile([KT, 1], f32)              # zero exp bias
        nc.vector.memset(zb_s, 0.0)
        ident = const.tile([128, 128], f32)
        kv_s = const.tile([128, T], f32r)             # rows 0:64 k^T, 64:128 v^T
        ve_s = const.tile([128, NKT * HE], f32r)      # key tile j at cols j*HE
        qt_s = const.tile([H, NB * QB], f32r)         # q^T, block i at cols i*QB

        make_identity(nc, ident)
        # Diagonal causal masks, generated on-chip: slot s keeps column c
        # of key-partition p iff c >= p + 128*s.
        nc.gpsimd.memset(mk_s, 1.0)
        for s in range(4):
            nc.gpsimd.affine_select(
                out=mk_s[:, s * QB:(s + 1) * QB],
                in_=mk_s[:, s * QB:(s + 1) * QB],
                compare_op=mybir.AluOpType.is_ge,
                fill=0.0,
                base=-128 * s,
                channel_multiplier=-1,
                pattern=[[1, QB]],
            )

        # ---- DMA issue (SP queue order == priority order) ----
        STREAM = [0, 2, 4, 6, 7, 1, 3, 5]
        xt_tiles = {}

        def issue_xt_dma(t, groups, lo=0, hi=None):
            """Issue chunk-range [lo, hi) of col-block t as `groups`-sized
            chunk-group DMAs."""
            if t not in xt_tiles:
                xt_tiles[t] = xt_pool.tile(
                    [128, DC * CB], f32r, tag="xt", name=f"xt{t}"
                )
            xt_t = xt_tiles[t]
            step = DC // groups
            for g in range(lo, hi if hi is not None else DC, step):
                nc.sync.dma_start(
                    out=xt_t.rearrange("p (d c) -> p d c", d=DC)[
                        :, g:g + step, :
                    ],
                    in_=xt_d.rearrange("(d p) t -> p d t", p=128)[
                        :, g:g + step, t * CB:(t + 1) * CB
                    ],
                )

        def issue_wkv_chunk(d0, d1):
            nc.sync.dma_start(
                out=wkv_s.rearrange("p (d h) -> p d h", d=DC)[:, d0:d1, :],
                in_=wkv_d.rearrange("(d p) h -> p d h", p=128)[:, d0:d1, :],
            )

        # Interleave the first weight chunk with the first x chunk so the
        # d-th KV matmul's inputs both land ~d*728ns in.
        issue_wkv_chunk(0, 1)
        issue_xt_dma(0, 8, lo=0, hi=1)
        nc.sync.dma_start(out=wq_s, in_=wq_d)  # host pre-laid-out [128, DC*H]
        issue_wkv_chunk(1, DC)
        issue_xt_dma(0, 8, lo=1, hi=2)
        nc.sync.dma_start(out=bias_s, in_=bias_d)
        issue_xt_dma(0, 8, lo=2)
        # Ones column of extended V (softmax denominator), strided into
        # every key tile's column H. memset can't target f32r tiles, so
        # the ones come from a tiny DRAM input.
        nc.sync.dma_start(
            out=ve_s.rearrange("p (j e) -> p j e", e=HE)[:, :, H:H + 1],
            in_=on_d.rearrange("p (j e) -> p j e", e=1),
        )
        issue_xt_dma(STREAM[1], 8)        # fine-grained: its trickle
        for t in STREAM[2:]:              # overlaps the diag0 phase
            issue_xt_dma(t, 4)

        # ---- attention micro-op machinery ----

        oacc = []
        for _i in range(NB):
            acc_tile = const.tile([HE, QB], f32, tag=f"oacc{_i}")
            oacc.append(acc_tile)
        first_drain = [True] * NB

        LOOK = 10
        pending = {i: [] for i in range(NB)}     # block -> [(j, pt), ...]

        def s_pair(i, j0, j1):
            """Emit two S matmuls into one 2-bank PSUM pair, ONE wide exp
            (-> diag mask on GPSIMD); both pts -> pending[i]."""
            psp = ps_s.tile([KT, 2 * QB], f32, tag="sp", name="psp")
            for half, j in enumerate((j0, j1)):
                nc.tensor.matmul(
                    psp[:, half * QB:(half + 1) * QB],
                    lhsT=kv_s[0:64, j * KT:(j + 1) * KT],
                    rhs=qt_s[:, i * QB:(i + 1) * QB],
                    start=True,
                    stop=True,
                )
            pt = ppool.tile([KT, 2 * QB], f32r)
            # Tail (wrap-around) pairs: parity-0 cores kill them with a
            # -1e30 pre-exp bias; parity-1 keeps them (bias 0).
            bias = tb_s if j0 >= 28 else zb_s
            nc.scalar.activation(
                pt, psp, AF.Exp, bias=bias, scale=float(D) ** -0.5
            )
            if 8 * i <= j0 < 8 * i + 4:
                half = (j0 - 8 * i) // 2             # diagonal mask pair
                nc.gpsimd.tensor_mul(
                    pt, pt,
                    mk_s[:, half * 2 * QB:(half + 1) * 2 * QB].bitcast(f32r)
                )
            pending[i].append((j0, pt[:, 0:QB]))
            pending[i].append((j1, pt[:, QB:2 * QB]))

        def attn_thunks(ops):
            """Build the phase's micro-op thunk list. ("blk", i, js, drain)
            AV-consumes block i's pending tiles plus newly S-emitted js as
            one PSUM group; ("pre", i, js) S-emits only (AVs land next
            phase). Thunks interleave S LOOK tiles ahead of new-tile AVs."""
            s_stream = []
            av_stream = []
            for op in ops:
                if op[0] == "pre":
                    _, i, js = op
                    s_stream.extend((i, j) for j in js)
                else:
                    _, i, js, drain = op
                    n_pend = len(pending[i])
                    s_stream.extend((i, j) for j in js)
                    av_stream.append((i, n_pend + len(js), n_pend, drain))
            thunks = []
            state = {"s": 0, "new": 0}

            def mk_s_thunk():
                k = state["s"]
                state["s"] += 2
                (i, j0), (i1, j1) = s_stream[k], s_stream[k + 1]
                assert i == i1
                return lambda: s_pair(i, j0, j1)

            def force_s():
                i, j0 = s_stream[state["s"]]
                _, j1 = s_stream[state["s"] + 1]
                s_pair(i, j0, j1)
                state["s"] += 2

            for i, n_tiles, n_pend0, drain in av_stream:
                po_box = []
                for t in range(n_tiles):
                    if t >= n_pend0:
                        state["new"] += 1
                    while state["s"] < len(s_stream) and (
                        state["s"] < state["new"] + LOOK
                    ):
                        thunks.append(mk_s_thunk())

                    def av_thunk(i=i, t=t, n_tiles=n_tiles, drain=drain,
                                 po_box=po_box):
                        while not pending[i]:
                            force_s()
                        if t == 0:
                            po_box.append(
                                ps_o.tile([HE, QB], f32, tag="po",
                                          name=f"po{i}"))
                        j, pt = pending[i].pop(0)
                        nc.tensor.matmul(
                            po_box[0],
                            lhsT=ve_s[:, j * HE:(j + 1) * HE],
                            rhs=pt,
                            start=(t == 0),
                            stop=(t == n_tiles - 1),
                        )
                        if drain and t == n_tiles - 1:
                            if first_drain[i]:
                                nc.vector.tensor_copy(oacc[i], po_box[0])
                                first_drain[i] = False
                            else:
                                nc.vector.tensor_add(oacc[i], oacc[i],
                                                     po_box[0])
                    thunks.append(av_thunk)
            while state["s"] < len(s_stream):
                thunks.append(mk_s_thunk())
            return thunks

        filler = []                                   # pending thunks

        def fill(n):
            for _ in range(min(n, len(filler))):
                filler.pop(0)()

        def stage_mm(t):
            """Projection matmuls for col t, interleaved with attention
            micro-ops so the PE fills the chunk-DMA gaps and the exp
            stream keeps flowing; then bias drains + v-transposes."""
            xt_t = xt_tiles[t]
            pkv = ps_a.tile([128, CB], f32, tag="pkv")
            pq = None
            if t % 2 == 0:
                pq = ps_qt.tile([128, CB], f32, tag="qt", name="pq")
            for d in range(DC):
                nc.tensor.matmul(
                    pkv,
                    lhsT=wkv_s[:, d * 128:(d + 1) * 128],
                    rhs=xt_t[:, d * CB:(d + 1) * CB],
                    start=(d == 0),
                    stop=(d == DC - 1),
                )
                if pq is not None:
                    nc.tensor.matmul(
                        pq[0:H, :],
                        lhsT=wq_s[:, d * H:(d + 1) * H],
                        rhs=xt_t[:, d * CB:(d + 1) * CB],
                        start=(d == 0),
                        stop=(d == DC - 1),
                    )
                fill(1 if pq is not None else 2)
            nc.vector.tensor_scalar_add(
                kv_s[:, t * CB:(t + 1) * CB], pkv, bkv_s
            )
            if pq is not None:
                i = t // 2
                nc.vector.tensor_scalar_add(
                    qt_s[:, i * QB:(i + 1) * QB], pq[0:H, :], bq_s
                )
            fill(len(filler))                         # flush rest
            for sub in range(4):                     # v^T -> natural-v tiles
                j = 4 * t + sub
                ptr = ps_qt.tile([128, CB], f32, tag="qt", name="ptr")
                nc.tensor.transpose(
                    ptr[:, 0:H],
                    kv_s[64:128, t * CB + sub * KT:t * CB + (sub + 1) * KT].bitcast(f32),
                    ident[64:128, 64:128],
                )
                nc.vector.tensor_copy(ve_s[:, j * HE:j * HE + H], ptr[:, 0:H])

        def attn_epi(i):
            for sub in range(4):
                ptr = ps_qt.tile([128, CB], f32, tag="qt", name="eptr")
                nc.tensor.transpose(
                    ptr[:, 0:HE], oacc[i][:, sub * 128:(sub + 1) * 128],
                    ident[0:HE, 0:HE]
                )
                rcp = opool.tile([128, 1], f32, tag="rcp")
                nc.vector.reciprocal(rcp, ptr[:, H:HE])
                if sub % 2 == 0:
                    ot2 = opool.tile([128, 2 * H], f32, tag="out")
                nc.vector.tensor_scalar_mul(
                    ot2[:, (sub % 2) * H:(sub % 2 + 1) * H], ptr[:, 0:H], rcp)
                if sub % 2 == 1:
                    nc.sync.dma_start(
                        out=o_d[i * 2 + (sub - 1) // 2], in_=ot2,
                    )

        def phase(ops):
            filler.extend(attn_thunks(ops))

        # Phase layout: each phase's thunks are queued before the next
        # stage_mm and interleave with its chunk trickle. Diag pieces are
        # S-emitted one phase early ("pre") so the GPSIMD mask-multiply
        # is covered; their AVs land next phase.
        stage_mm(0)
        phase([("pre", 0, [0, 1, 2, 3])])
        fill(2)
        stage_mm(2)
        phase([("blk", 0, [], True),
               ("blk", 1, [0, 1, 2, 3], True),
               ("pre", 1, [8, 9, 10, 11])])
        stage_mm(4)
        phase([("blk", 1, [], True),
               ("blk", 2, [0, 1, 2, 3, 8, 9, 10, 11], True),
               ("pre", 2, [16, 17, 18, 19])])
        stage_mm(6)
        phase([("blk", 2, [], True),
               ("blk", 3, [0, 1, 2, 3], True),
               ("pre", 3, [24, 25, 26, 27])])
        stage_mm(7)
        phase([("blk", 3, [8, 9, 10, 11, 16, 17, 18, 19], True),
               ("blk", 0, [28, 29, 30, 31], True)])
        stage_mm(1)
        attn_epi(0)
        phase([("blk", 1, [28, 29, 30, 31, 4, 5, 6, 7], True),
               ("blk", 2, [28, 29, 30, 31, 4, 5, 6, 7], True)])
        stage_mm(3)
        attn_epi(1)
        phase([("blk", 2, [12, 13, 14, 15], True),
               ("blk", 3, [28, 29, 30, 31, 4, 5, 6, 7,
                           12, 13, 14, 15], True)])
        stage_mm(5)
        attn_epi(2)
        phase([("blk", 3, [20, 21, 22, 23], True)])
        fill(len(filler))
        attn_epi(3)
        assert not filler and all(not v for v in pending.values())

    nc.compile()
    return nc


def _get_program():
    global _PROGRAM
    if _PROGRAM is None:
        _PROGRAM = _build_program()
    return _PROGRAM


def build_in_maps(inputs):
    x = np.asarray(inputs["x"], np.float32)
    wq = np.asarray(inputs["Wq"], np.float32)            # [D, H]
    # host re-layout to [128, DC*H]: chunk d (rows d*128..) at cols d*H
    wq = np.ascontiguousarray(
        wq.reshape(DC, 128, H).transpose(1, 0, 2).reshape(128, DC * H)
    )
    wkv = np.ascontiguousarray(
        np.concatenate(
            [np.asarray(inputs["Wk"], np.float32),
             np.asarray(inputs["Wv"], np.float32)], axis=1
        )
    )
    bias = np.zeros((2, 128, 3), np.float32)
    for p in range(2):
        bias[p, 0:H, 0] = np.asarray(inputs["bq"], np.float32)
        bias[p, 0:H, 1] = np.asarray(inputs["bk"], np.float32)
        bias[p, H:2 * H, 1] = np.asarray(inputs["bv"], np.float32)
        bias[p, :, 2] = 0.0 if p == 1 else -1e30
    in_maps = []
    for core in range(NCORES):
        b, p = core // 2, core % 2
        xt = x[b].T
        if p:
            xt = np.roll(xt, -512, axis=1)
        in_maps.append({
            "xt": np.ascontiguousarray(xt),
            "wq": wq,
            "wkv": wkv,
            "bias": np.ascontiguousarray(bias[p]),
            "ones": np.ones((128, NKT), np.float32),
        })
    return in_maps


def assemble_out(results):
    out = np.empty((B, T, H), np.float32)
    for core in range(NCORES):
        b, p = core // 2, core % 2
        o = np.asarray(results[core]["o"])    # [NB*2, 128, 2H]
        o = o.reshape(NB * 2, 128, 2, H).transpose(0, 2, 1, 3).reshape(NB, QB, H)
        for i in range(NB):
            g = 1024 * i + 512 * p
            out[b, g:g + QB] = o[i]
    return out


def kernel(**inputs):
    from concourse.bass_utils import run_bass_kernel_spmd

    nc = _get_program()
    in_maps = build_in_maps(inputs)
    res = run_bass_kernel_spmd(nc, in_maps, list(range(NCORES)))
    return assemble_out(res.results)


# revision 50
# speedup vs baseline: 1.0066x; 1.0066x over previous
"""Causal single-head attention on 8 TRN2 NeuronCores (Bass/Tile).

Problem: x[4,4096,1024] @ {Wq,Wk,Wv}[1024,64] (+zero biases) -> causal
softmax attention -> out[4,4096,64], fp32.

Sharding: 8 cores = 4 batches x 2 parities. Each core owns 4 query
blocks of 512 rows. Parity-1 cores receive x^T rolled left by 512
columns so every core's query blocks sit at uniform offsets 1024*i,
keeping the program SPMD-identical; causality is enforced by 4
data-driven diagonal mask tiles plus a parity-dependent pre-exp bias
(-1e30 kills the wrap-around key tiles on parity-0 cores).

Softmax uses no max-subtraction (scores ~N(0,0.25^2), exp is safe) and
the denominator comes from a ones-column appended to V, so there are no
cross-partition reductions. All matmuls run as float32r (full PE rate
at N>=256 moving dim; rel err ~2e-4).

Schedule: all x^T column-block DMAs are issued up-front (SP queue runs
them back-to-back at full DMA rate) in stream order 0,2,4,6,7,1,3,5;
blocks arrive in 1-2 chunk pieces. Attention work is queued as micro-op
thunks (one S->exp->mask or one AV each) and interleaved between the
projection chunk matmuls, so the PE fills the 515ns DMA-trickle gaps
and the ScalarE exp stream never starves during a stream window. The S
matmul runs LOOK=3 key-tiles ahead of the AV accumulation; diagonal
tiles are S-emitted one phase early so their GPSIMD mask-multiply is
off the critical path. PSUM banks: pkv 1 + shared{pq,transpose} 2 +
S 4 + attention-out po 2 = 8 (projection and transpose PSUM share
one 2-buffer pool: their lifetimes chain through the same kv_s drain).
"""

import numpy as np

B, T, D, H = 4, 4096, 1024, 64
NCORES = 8
QB = 512          # query block width (free dim of attention matmuls)
KT = 128          # key tile (partition dim of P^T)
DC = D // 128     # 8 contraction chunks
CB = 512          # x^T column block for streaming
NCB = T // CB     # 8
NKT = T // KT     # 32
NB = 4            # query blocks per core
HE = H + 1        # V extended with a ones column (softmax denominator)

_PROGRAM = None


def _build_program():
    from contextlib import ExitStack

    import concourse.bass as bass  # noqa: F401
    import concourse.mybir as mybir
    import concourse.tile as tile
    from concourse import bacc
    from concourse.masks import make_identity

    f32 = mybir.dt.float32
    f32r = mybir.dt.float32r
    AF = mybir.ActivationFunctionType

    nc = bacc.Bacc(target_bir_lowering=False)
    xt_d = nc.dram_tensor("xt", [D, T], f32r, kind="ExternalInput").ap()
    wq_d = nc.dram_tensor("wq", [128, DC * H], f32r, kind="ExternalInput").ap()
    wkv_d = nc.dram_tensor("wkv", [D, 2 * H], f32r, kind="ExternalInput").ap()
    # col 0 rows 0:64 = bq, col 1 = bkv, col 2 = tail bias (0 / -1e30)
    bias_d = nc.dram_tensor("bias", [128, 3], f32, kind="ExternalInput").ap()
    on_d = nc.dram_tensor("ones", [128, NKT], f32r, kind="ExternalInput").ap()
    o_d = nc.dram_tensor("o", [NB * 2, 128, 2 * H], f32, kind="ExternalOutput").ap()

    with ExitStack() as ctx:
        tc = ctx.enter_context(tile.TileContext(nc))
        const = ctx.enter_context(tc.tile_pool(name="const", bufs=1))
        xt_pool = ctx.enter_context(tc.tile_pool(name="xtp", bufs=5))
        ppool = ctx.enter_context(tc.tile_pool(name="ptp", bufs=16))
        opool = ctx.enter_context(tc.tile_pool(name="otp", bufs=2))
        ps_a = ctx.enter_context(tc.tile_pool(name="psA", bufs=1, space="PSUM"))
        ps_qt = ctx.enter_context(tc.tile_pool(name="psQT", bufs=2, space="PSUM"))
        ps_s = ctx.enter_context(tc.tile_pool(name="psS", bufs=2, space="PSUM"))
        ps_o = ctx.enter_context(tc.tile_pool(name="psO", bufs=1, space="PSUM"))

        # Persistent SBUF state
        wq_s = const.tile([128, DC * H], f32r)        # chunk d at cols d*H
        wkv_s = const.tile([128, DC * 2 * H], f32r)   # chunk d at cols d*2H
        bias_s = const.tile([128, 3], f32)
        bq_s = bias_s[0:H, 0:1]
        bkv_s = bias_s[:, 1:2]
        tb_s = bias_s[:, 2:3]                        # tail-tile exp bias
        mk_s = const.tile([KT, 4 * QB], f32)          # mask slot s at cols s*QB
        zb_s = const.tile([KT, 1], f32)              # zero exp bias
        nc.vector.memset(zb_s, 0.0)
        ident = const.tile([128, 128], f32)
        kv_s = const.tile([128, T], f32r)             # rows 0:64 k^T, 64:128 v^T
        ve_s = const.tile([128, NKT * HE], f32r)      # key tile j at cols j*HE
        qt_s = const.tile([H, NB * QB], f32r)         # q^T, block i at cols i*QB

        make_identity(nc, ident)
        # PE p-state warmup: ~3.5us of junk transposes so the 3us clock
        # ramp completes before the first real projection matmuls.
        for _w in range(16):
            jt = ps_qt.tile([128, CB], f32, tag="qt", name="warm")
            nc.tensor.transpose(jt[:, 0:128], ident, ident)
        # Diagonal causal masks, generated on-chip: slot s keeps column c
        # of key-partition p iff c >= p + 128*s.
        nc.gpsimd.memset(mk_s, 1.0)
        for s in range(4):
            nc.gpsimd.affine_select(
                out=mk_s[:, s * QB:(s + 1) * QB],
                in_=mk_s[:, s * QB:(s + 1) * QB],
                compare_op=mybir.AluOpType.is_ge,
                fill=0.0,
                base=-128 * s,
                channel_multiplier=-1,
                pattern=[[1, QB]],
            )

        # ---- DMA issue (SP queue order == priority order) ----
        STREAM = [0, 2, 4, 6, 7, 1, 3, 5]
        xt_tiles = {}

        def issue_xt_dma(t, groups, lo=0, hi=None):
            """Issue chunk-range [lo, hi) of col-block t as `groups`-sized
            chunk-group DMAs."""
            if t not in xt_tiles:
                xt_tiles[t] = xt_pool.tile(
                    [128, DC * CB], f32r, tag="xt", name=f"xt{t}"
                )
            xt_t = xt_tiles[t]
            step = DC // groups
            for g in range(lo, hi if hi is not None else DC, step):
                nc.sync.dma_start(
                    out=xt_t.rearrange("p (d c) -> p d c", d=DC)[
                        :, g:g + step, :
                    ],
                    in_=xt_d.rearrange("(d p) t -> p d t", p=128)[
                        :, g:g + step, t * CB:(t + 1) * CB
                    ],
                )

        def issue_wkv_chunk(d0, d1):
            nc.sync.dma_start(
                out=wkv_s.rearrange("p (d h) -> p d h", d=DC)[:, d0:d1, :],
                in_=wkv_d.rearrange("(d p) h -> p d h", p=128)[:, d0:d1, :],
            )

        # Interleave the first weight chunk with the first x chunk so the
        # d-th KV matmul's inputs both land ~d*728ns in.
        issue_wkv_chunk(0, 1)
        issue_xt_dma(0, 8, lo=0, hi=1)
        nc.sync.dma_start(out=wq_s, in_=wq_d)  # host pre-laid-out [128, DC*H]
        issue_wkv_chunk(1, DC)
        issue_xt_dma(0, 8, lo=1, hi=2)
        nc.sync.dma_start(out=bias_s, in_=bias_d)
        issue_xt_dma(0, 8, lo=2)
        # Ones column of extended V (softmax denominator), strided into
        # every key tile's column H. memset can't target f32r tiles, so
        # the ones come from a tiny DRAM input.
        nc.sync.dma_start(
            out=ve_s.rearrange("p (j e) -> p j e", e=HE)[:, :, H:H + 1],
            in_=on_d.rearrange("p (j e) -> p j e", e=1),
        )
        for t in STREAM[1:]:
            issue_xt_dma(t, 4)

        # ---- attention micro-op machinery ----

        oacc = []
        for _i in range(NB):
            acc_tile = const.tile([HE, QB], f32, tag=f"oacc{_i}")
            oacc.append(acc_tile)
        first_drain = [True] * NB

        LOOK = 10
        pending = {i: [] for i in range(NB)}     # block -> [(j, pt), ...]

        def s_pair(i, j0, j1):
            """Emit two S matmuls into one 2-bank PSUM pair, ONE wide exp
            (-> diag mask on GPSIMD); both pts -> pending[i]."""
            psp = ps_s.tile([KT, 2 * QB], f32, tag="sp", name="psp")
            for half, j in enumerate((j0, j1)):
                nc.tensor.matmul(
                    psp[:, half * QB:(half + 1) * QB],
                    lhsT=kv_s[0:64, j * KT:(j + 1) * KT],
                    rhs=qt_s[:, i * QB:(i + 1) * QB],
                    start=True,
                    stop=True,
                )
            pt = ppool.tile([KT, 2 * QB], f32r)
            # Tail (wrap-around) pairs: parity-0 cores kill them with a
            # -1e30 pre-exp bias; parity-1 keeps them (bias 0).
            bias = tb_s if j0 >= 28 else zb_s
            nc.scalar.activation(
                pt, psp, AF.Exp, bias=bias, scale=float(D) ** -0.5
            )
            if 8 * i <= j0 < 8 * i + 4:
                half = (j0 - 8 * i) // 2             # diagonal mask pair
                nc.gpsimd.tensor_mul(
                    pt, pt,
                    mk_s[:, half * 2 * QB:(half + 1) * 2 * QB].bitcast(f32r)
                )
            pending[i].append((j0, pt[:, 0:QB]))
            pending[i].append((j1, pt[:, QB:2 * QB]))

        def attn_thunks(ops):
            """Build the phase's micro-op thunk list. ("blk", i, js, drain)
            AV-consumes block i's pending tiles plus newly S-emitted js as
            one PSUM group; ("pre", i, js) S-emits only (AVs land next
            phase). Thunks interleave S LOOK tiles ahead of new-tile AVs."""
            s_stream = []
            av_stream = []
            for op in ops:
                if op[0] == "pre":
                    _, i, js = op
                    s_stream.extend((i, j) for j in js)
                else:
                    _, i, js, drain = op
                    n_pend = len(pending[i])
                    s_stream.extend((i, j) for j in js)
                    av_stream.append((i, n_pend + len(js), n_pend, drain))
            thunks = []
            state = {"s": 0, "new": 0}

            def mk_s_thunk():
                k = state["s"]
                state["s"] += 2
                (i, j0), (i1, j1) = s_stream[k], s_stream[k + 1]
                assert i == i1
                return lambda: s_pair(i, j0, j1)

            def force_s():
                i, j0 = s_stream[state["s"]]
                _, j1 = s_stream[state["s"] + 1]
                s_pair(i, j0, j1)
                state["s"] += 2

            for i, n_tiles, n_pend0, drain in av_stream:
                po_box = []
                for t in range(n_tiles):
                    if t >= n_pend0:
                        state["new"] += 1
                    while state["s"] < len(s_stream) and (
                        state["s"] < state["new"] + LOOK
                    ):
                        thunks.append(mk_s_thunk())

                    def av_thunk(i=i, t=t, n_tiles=n_tiles, drain=drain,
                                 po_box=po_box):
                        while not pending[i]:
                            force_s()
                        if t == 0:
                            po_box.append(
                                ps_o.tile([HE, QB], f32, tag="po",
                                          name=f"po{i}"))
                        j, pt = pending[i].pop(0)
                        nc.tensor.matmul(
                            po_box[0],
                            lhsT=ve_s[:, j * HE:(j + 1) * HE],
                            rhs=pt,
                            start=(t == 0),
                            stop=(t == n_tiles - 1),
                        )
                        if drain and t == n_tiles - 1:
                            if first_drain[i]:
                                nc.vector.tensor_copy(oacc[i], po_box[0])
                                first_drain[i] = False
                            else:
                                nc.vector.tensor_add(oacc[i], oacc[i],
                                                     po_box[0])
                    thunks.append(av_thunk)
            while state["s"] < len(s_stream):
                thunks.append(mk_s_thunk())
            return thunks

        filler = []                                   # pending thunks

        def fill(n):
            for _ in range(min(n, len(filler))):
                filler.pop(0)()

        def stage_mm(t):
            """Projection matmuls for col t, interleaved with attention
            micro-ops so the PE fills the chunk-DMA gaps and the exp
            stream keeps flowing; then bias drains + v-transposes."""
            xt_t = xt_tiles[t]
            pkv = ps_a.tile([128, CB], f32, tag="pkv")
            pq = None
            if t % 2 == 0:
                pq = ps_qt.tile([128, CB], f32, tag="qt", name="pq")
            for d in range(DC):
                nc.tensor.matmul(
                    pkv,
                    lhsT=wkv_s[:, d * 128:(d + 1) * 128],
                    rhs=xt_t[:, d * CB:(d + 1) * CB],
                    start=(d == 0),
                    stop=(d == DC - 1),
                )
                if pq is not None:
                    nc.tensor.matmul(
                        pq[0:H, :],
                        lhsT=wq_s[:, d * H:(d + 1) * H],
                        rhs=xt_t[:, d * CB:(d + 1) * CB],
                        start=(d == 0),
                        stop=(d == DC - 1),
                    )
                fill(1 if pq is not None else 2)
            nc.vector.tensor_scalar_add(
                kv_s[:, t * CB:(t + 1) * CB], pkv, bkv_s
            )
            if pq is not None:
                i = t // 2
                nc.vector.tensor_scalar_add(
                    qt_s[:, i * QB:(i + 1) * QB], pq[0:H, :], bq_s
                )
            fill(len(filler))                         # flush rest
            for sub in range(4):                     # v^T -> natural-v tiles
                j = 4 * t + sub
                ptr = ps_qt.tile([128, CB], f32, tag="qt", name="ptr")
                nc.tensor.transpose(
                    ptr[:, 0:H],
                    kv_s[64:128, t * CB + sub * KT:t * CB + (sub + 1) * KT].bitcast(f32),
                    ident[64:128, 64:128],
                )
                nc.vector.tensor_copy(ve_s[:, j * HE:j * HE + H], ptr[:, 0:H])

        def attn_epi(i):
            for sub in range(4):
                ptr = ps_qt.tile([128, CB], f32, tag="qt", name="eptr")
                nc.tensor.transpose(
                    ptr[:, 0:HE], oacc[i][:, sub * 128:(sub + 1) * 128],
                    ident[0:HE, 0:HE]
                )
                rcp = opool.tile([128, 1], f32, tag="rcp")
                nc.vector.reciprocal(rcp, ptr[:, H:HE])
                if sub % 2 == 0:
                    ot2 = opool.tile([128, 2 * H], f32, tag="out")
                nc.vector.tensor_scalar_mul(
                    ot2[:, (sub % 2) * H:(sub % 2 + 1) * H], ptr[:, 0:H], rcp)
                if sub % 2 == 1:
                    nc.sync.dma_start(
                        out=o_d[i * 2 + (sub - 1) // 2], in_=ot2,
                    )

        def phase(ops):
            filler.extend(attn_thunks(ops))

        # Phase layout: each phase's thunks are queued before the next
        # stage_mm and interleave with its chunk trickle. Diag pieces are
        # S-emitted one phase early ("pre") so the GPSIMD mask-multiply
        # is covered; their AVs land next phase.
        stage_mm(0)
        phase([("pre", 0, [0, 1, 2, 3])])
        fill(2)
        stage_mm(2)
        phase([("blk", 0, [], True),
               ("blk", 1, [0, 1, 2, 3], True),
               ("pre", 1, [8, 9, 10, 11])])
        stage_mm(4)
        phase([("blk", 1, [], True),
               ("blk", 2, [0, 1, 2, 3, 8, 9, 10, 11], True),
               ("pre", 2, [16, 17, 18, 19])])
        stage_mm(6)
        phase([("blk", 2, [], True),
               ("blk", 3, [0, 1, 2, 3], True),
               ("pre", 3, [24, 25, 26, 27])])
        stage_mm(7)
        phase([("blk", 3, [8, 9, 10, 11, 16, 17, 18, 19], True),
               ("blk", 0, [28, 29, 30, 31], True)])
        stage_mm(1)
        attn_epi(0)
        phase([("blk", 1, [28, 29, 30, 31, 4, 5, 6, 7], True),
               ("blk", 2, [28, 29, 30, 31, 4, 5, 6, 7], True)])
        stage_mm(3)
        attn_epi(1)
        phase([("blk", 2, [12, 13, 14, 15], True),
               ("blk", 3, [28, 29, 30, 31, 4, 5, 6, 7,
                           12, 13, 14, 15], True)])
        stage_mm(5)
        attn_epi(2)
        phase([("blk", 3, [20, 21, 22, 23], True)])
        fill(len(filler))
        attn_epi(3)
        assert not filler and all(not v for v in pending.values())

    nc.compile()
    return nc


def _get_program():
    global _PROGRAM
    if _PROGRAM is None:
        _PROGRAM = _build_program()
    return _PROGRAM


def build_in_maps(inputs):
    x = np.asarray(inputs["x"], np.float32)
    wq = np.asarray(inputs["Wq"], np.float32)            # [D, H]
    # host re-layout to [128, DC*H]: chunk d (rows d*128..) at cols d*H
    wq = np.ascontiguousarray(
        wq.reshape(DC, 128, H).transpose(1, 0, 2).reshape(128, DC * H)
    )
    wkv = np.ascontiguousarray(
        np.concatenate(
            [np.asarray(inputs["Wk"], np.float32),
             np.asarray(inputs["Wv"], np.float32)], axis=1
        )
    )
    bias = np.zeros((2, 128, 3), np.float32)
    for p in range(2):
        bias[p, 0:H, 0] = np.asarray(inputs["bq"], np.float32)
        bias[p, 0:H, 1] = np.asarray(inputs["bk"], np.float32)
        bias[p, H:2 * H, 1] = np.asarray(inputs["bv"], np.float32)
        bias[p, :, 2] = 0.0 if p == 1 else -1e30
    in_maps = []
    for core in range(NCORES):
        b, p = core // 2, core % 2
        xt = x[b].T
        if p:
            xt = np.roll(xt, -512, axis=1)
        in_maps.append({
            "xt": np.ascontiguousarray(xt),
            "wq": wq,
            "wkv": wkv,
            "bias": np.ascontiguousarray(bias[p]),
            "ones": np.ones((128, NKT), np.float32),
        })
    return in_maps


def assemble_out(results):
    out = np.empty((B, T, H), np.float32)
    for core in range(NCORES):
        b, p = core // 2, core % 2
        o = np.asarray(results[core]["o"])    # [NB*2, 128, 2H]
        o = o.reshape(NB * 2, 128, 2, H).transpose(0, 2, 1, 3).reshape(NB, QB, H)
        for i in range(NB):
            g = 1024 * i + 512 * p
            out[b, g:g + QB] = o[i]
    return out


def kernel(**inputs):
    from concourse.bass_utils import run_bass_kernel_spmd

    nc = _get_program()
    in_maps = build_in_maps(inputs)
    res = run_bass_kernel_spmd(nc, in_maps, list(range(NCORES)))
    return assemble_out(res.results)
